# revision 1
# baseline (speedup 1.0000x reference)
"""Causal depthwise conv (kernel_size=4) on 8 TRN2 NeuronCores.

Problem: x (4, 4096, 16, 128) f32, weight (4, 16, 128) f32,
out[b,t,h,d] = sum_k weight[k,h,d] * x[b,t-k,h,d]   (zero-pad t<0).

Sharding: tensor-parallel over heads — core c owns heads [2c, 2c+2).
Host transposes each core's slice to d-major layout so that on-device the
partition dim is d (128) and the free dim is t. Then weight[k,h,:] is a
per-partition scalar and the whole conv per (h, b) stream is:

    acc = w0 * x                          (ScalarE activation, scale=w0)
    acc = (x >> k) * wk + acc, k=1..3     (fused scalar_tensor_tensor)

Each stream ships with 3 zero columns prepended (causal pad), so taps never
cross stream boundaries and every DMA row is a single contiguous 16 KB run.
ScalarE handles the k=0 multiply, VectorE the 3 fused multiply-accumulate
taps; both overlap the HBM DMA stream, which is the binding resource
(~100 us/core for 33.6 MB; measured kernel ~106 us).
"""

import time

import numpy as np

import concourse.mybir as mybir
from concourse import bacc, tile
from concourse.bass_utils import run_bass_kernel_spmd

BATCH, SEQ, N_HEADS, D_HEAD = 4, 4096, 16, 128
KERNEL = 4
PAD = KERNEL - 1
N_CORES = 8
H_PER_CORE = N_HEADS // N_CORES          # 2
N_STREAMS = H_PER_CORE * BATCH           # 8 per core; stream j = hl*BATCH + b

F32 = mybir.dt.float32

PROFILE = False          # set by test.py; adds a profiled run
TRACE_KWARGS = {}
last_exec_time_ns = None
last_results = None


def _build_module(chain: bool = False, repeats: int = 1):
    """chain=True builds the timing variant: out has the same shape as x
    (pad columns written as zeros) so outputs can feed back as inputs for
    device-resident repeated-execution timing. repeats>1 runs the whole
    kernel body that many times inside one NEFF (timing only)."""
    nc = bacc.Bacc(
        "TRN2",
        target_bir_lowering=False,
        debug=False,
        num_devices=N_CORES,
        enable_asserts=False,
    )
    out_t = SEQ + PAD if chain else SEQ
    x = nc.dram_tensor("x", [D_HEAD, N_STREAMS, SEQ + PAD], F32, kind="ExternalInput").ap()
    w = nc.dram_tensor("w", [D_HEAD, H_PER_CORE * KERNEL], F32, kind="ExternalInput").ap()
    out = nc.dram_tensor("out", [D_HEAD, N_STREAMS, out_t], F32, kind="ExternalOutput").ap()
    pad_off = PAD if chain else 0

    with tile.TileContext(nc) as tc:
        with (
            tc.tile_pool(name="wp", bufs=1) as wp,
            tc.tile_pool(name="xp", bufs=6) as xp,
            tc.tile_pool(name="dp", bufs=4) as dp,
        ):
            wt = wp.tile([D_HEAD, H_PER_CORE * KERNEL], F32)
            nc.sync.dma_start(out=wt, in_=w)
            if chain:
                zt = wp.tile([D_HEAD, N_STREAMS * PAD], F32)
                nc.vector.memset(zt, 0.0)
                nc.sync.dma_start(
                    out=out[:, :, 0:PAD], in_=zt[:, :].rearrange("p (j q) -> p j q", q=PAD)
                )
            for _r in range(repeats):
                for j in range(N_STREAMS):
                    hl = j // BATCH
                    X = xp.tile([D_HEAD, SEQ + PAD], F32, tag="x")
                    nc.sync.dma_start(out=X, in_=x[:, j, :])
                    acc = dp.tile([D_HEAD, SEQ], F32, tag="acc")
                    w0 = wt[:, hl * KERNEL : hl * KERNEL + 1]
                    nc.scalar.activation(
                        acc, X[:, PAD : PAD + SEQ],
                        mybir.ActivationFunctionType.Copy, scale=w0,
                    )
                    for k in range(1, KERNEL):
                        wk = wt[:, hl * KERNEL + k : hl * KERNEL + k + 1]
                        nc.vector.scalar_tensor_tensor(
                            acc, X[:, PAD - k : PAD + SEQ - k], wk, acc,
                            mybir.AluOpType.mult, mybir.AluOpType.add,
                        )
                    nc.sync.dma_start(out=out[:, j, pad_off : pad_off + SEQ], in_=acc)
    nc.compile()
    return nc


_module = None


def _get_module():
    global _module
    if _module is None:
        _module = _build_module()
    return _module


def _shard_inputs(x: np.ndarray, weight: np.ndarray):
    in_maps = []
    for c in range(N_CORES):
        h0 = c * H_PER_CORE
        xs = x[:, :, h0 : h0 + H_PER_CORE, :]            # (B, T, HL, D)
        xt = np.ascontiguousarray(xs.transpose(3, 2, 0, 1))  # (D, HL, B, T)
        xin = np.zeros((D_HEAD, N_STREAMS, SEQ + PAD), dtype=np.float32)
        xin[:, :, PAD:] = xt.reshape(D_HEAD, N_STREAMS, SEQ)
        ws = weight[:, h0 : h0 + H_PER_CORE, :]          # (K, HL, D)
        warr = np.ascontiguousarray(ws.transpose(2, 1, 0)).reshape(D_HEAD, H_PER_CORE * KERNEL)
        in_maps.append({"x": xin, "w": warr.astype(np.float32)})
    return in_maps


def _unshard(results) -> np.ndarray:
    out = np.empty((BATCH, SEQ, N_HEADS, D_HEAD), dtype=np.float32)
    for c in range(N_CORES):
        h0 = c * H_PER_CORE
        o = results[c]["out"].reshape(D_HEAD, H_PER_CORE, BATCH, SEQ)
        out[:, :, h0 : h0 + H_PER_CORE, :] = o.transpose(2, 3, 1, 0)
    return out


def kernel(x: np.ndarray, weight: np.ndarray) -> np.ndarray:
    global last_exec_time_ns, last_results
    x = np.asarray(x, dtype=np.float32)
    weight = np.asarray(weight, dtype=np.float32)
    nc = _get_module()
    in_maps = _shard_inputs(x, weight)
    # The shared terminal occasionally wedges (NRT_EXEC_UNIT_UNRECOVERABLE)
    # and recovers after a pause; retry rather than fail the whole call.
    last_err = None
    for attempt in range(3):
        try:
            res = run_bass_kernel_spmd(
                nc, in_maps, list(range(N_CORES)), trace=PROFILE, **TRACE_KWARGS
            )
            break
        except Exception as e:  # noqa: BLE001 - device-transient errors
            last_err = e
            time.sleep(25 * (attempt + 1))
    else:
        raise last_err
    last_exec_time_ns = res.exec_time_ns
    last_results = res
    return _unshard(res.results)



# revision 3
# speedup vs baseline: 1.2872x; 1.2872x over previous
"""Causal depthwise conv (kernel_size=4) on 8 TRN2 NeuronCores — fp16 hybrid.

Problem: x (4, 4096, 16, 128) f32, weight (4, 16, 128) f32,
out[b,t,h,d] = sum_k weight[k,h,d] * x[b,t-k,h,d]   (zero-pad t<0).

Sharding: tensor-parallel over heads — core c owns heads [2c, 2c+2), giving
8 streams per core (stream j = hl*BATCH + b), each a [d=128, t=4096] lane.

The kernel is HBM-DMA-bound, so all device I/O is fp16 (host casts both
ways; rel-err ~1e-3 vs the 2e-2 gate): 16.8 MB per core, ~39 us at the
435 GB/s SBUF-fabric rate. Compute is split so every engine stays under
that bound:

  PE streams (5): each tap k is a matmul with a 128x128 *diagonal*
    stationary diag(w[k,h,:]) against the shifted moving slice x[d, t-k],
    accumulating all 4 taps into PSUM (f32) per 512-col bank; ScalarE
    evicts 2048-col spans PSUM->SBUF as fp16. PE ~34 us, ACT ~20 us.
  DVE streams (3): acc = x*w0 (tensor_scalar), then 3 fused
    scalar_tensor_tensor taps. Even-offset taps pack 2x; odd offsets run
    1x. ~32 us.

Input DMAs ride the SP HWDGE ring, output DMAs the ACT ring, so stores
never block loads in FIFO order. Each stream ships with 4 leading zero
columns (causal pad + even alignment), so taps never cross stream
boundaries and every DMA row is a contiguous 8.2 KB run.
"""

import time

import numpy as np

import concourse.mybir as mybir
from concourse import bacc, tile
from concourse.bass_utils import run_bass_kernel_spmd

BATCH, SEQ, N_HEADS, D_HEAD = 4, 4096, 16, 128
KERNEL = 4
PAD = 4                                   # causal pad (>=KERNEL-1), even
N_CORES = 8
H_PER_CORE = N_HEADS // N_CORES           # 2
N_STREAMS = H_PER_CORE * BATCH            # 8 per core; stream j = hl*BATCH + b
S = SEQ + PAD                             # per-stream stride in the flat layout
SLACK = 4                                 # trailing zero cols so shifted reads stay in-tile

DVE_STREAMS = (1, 3, 5)                   # interleaved with PE streams by DMA arrival
PE_STREAMS = tuple(j for j in range(N_STREAMS) if j not in DVE_STREAMS)
BANK = 512                                # one PSUM bank in f32 cols
EVICT = 2048                              # ACT eviction span (4 banks)

F32 = mybir.dt.float32
F16 = mybir.dt.float16

PROFILE = False          # set by test.py; adds a profiled run
TRACE_KWARGS = {}
last_exec_time_ns = None
last_results = None


def _build_module(chain: bool = False, repeats: int = 1):
    """repeats>1 runs the whole kernel body that many times inside one NEFF
    (timing only). `chain` is accepted for test.py compat (unused)."""
    nc = bacc.Bacc(
        "TRN2",
        target_bir_lowering=False,
        debug=False,
        num_devices=N_CORES,
        enable_asserts=False,
    )
    x = nc.dram_tensor("x", [D_HEAD, N_STREAMS * S + SLACK], F16, kind="ExternalInput").ap()
    # per-(hl,k) 128x128 diagonal stationaries, flattened on the free axis
    wd = nc.dram_tensor(
        "wd", [D_HEAD, H_PER_CORE * KERNEL * D_HEAD], F16, kind="ExternalInput"
    ).ap()
    # per-partition scalars w[d, hl*KERNEL + k] for the DVE taps
    ws = nc.dram_tensor("ws", [D_HEAD, H_PER_CORE * KERNEL], F32, kind="ExternalInput").ap()
    out = nc.dram_tensor("out", [D_HEAD, N_STREAMS, SEQ], F16, kind="ExternalOutput").ap()

    with tile.TileContext(nc) as tc:
        with (
            tc.tile_pool(name="wp", bufs=1) as wp,
            tc.tile_pool(name="xp", bufs=1) as xp,
            tc.tile_pool(name="accp", bufs=2) as accp,
            tc.tile_pool(name="outp", bufs=1) as outp,
            tc.psum_pool(name="pp", bufs=2) as pp,
        ):
            wdt = wp.tile([D_HEAD, H_PER_CORE * KERNEL * D_HEAD], F16)
            wst = wp.tile([D_HEAD, H_PER_CORE * KERNEL], F32)
            nc.sync.dma_start(out=wdt, in_=wd)
            nc.sync.dma_start(out=wst, in_=ws)

            for _r in range(repeats):
                X = xp.tile([D_HEAD, N_STREAMS * S + SLACK], F16, tag="x")
                # per-stream input DMAs so compute chases the load stream
                for j in range(N_STREAMS):
                    lo = j * S
                    hi = (j + 1) * S + (SLACK if j == N_STREAMS - 1 else 0)
                    nc.sync.dma_start(out=X[:, lo:hi], in_=x[:, lo:hi])

                # SBUF landing area for PE-stream results (indexed by position
                # in PE_STREAMS), fp16
                ope = outp.tile([D_HEAD, len(PE_STREAMS) * SEQ], F16, tag="ope")

                dve_done = 0
                for pi, j in enumerate(PE_STREAMS):
                    hl = j // BATCH
                    for half in range(SEQ // EVICT):
                        ps = pp.tile([D_HEAD, EVICT], F32, tag="ps")
                        for k in range(KERNEL):
                            wk = wdt[:, (hl * KERNEL + k) * D_HEAD : (hl * KERNEL + k + 1) * D_HEAD]
                            for c in range(EVICT // BANK):
                                base = j * S + PAD - k + half * EVICT + c * BANK
                                nc.tensor.matmul(
                                    ps[:, c * BANK : (c + 1) * BANK],
                                    wk,
                                    X[:, base : base + BANK],
                                    start=(k == 0),
                                    stop=(k == KERNEL - 1),
                                )
                        osb = ope[:, pi * SEQ + half * EVICT : pi * SEQ + (half + 1) * EVICT]
                        nc.scalar.activation(
                            osb, ps, mybir.ActivationFunctionType.Copy, scale=1.0
                        )
                        nc.scalar.dma_start(
                            out=out[:, j, half * EVICT : (half + 1) * EVICT], in_=osb
                        )
                    # interleave one DVE stream after each PE stream finishes
                    # issuing, so DVE work starts as soon as its data lands
                    if dve_done < len(DVE_STREAMS):
                        dj = DVE_STREAMS[dve_done]
                        dve_done += 1
                        hl = dj // BATCH
                        acc = accp.tile([D_HEAD, SEQ], F16, tag="acc")
                        w0 = wst[:, hl * KERNEL : hl * KERNEL + 1]
                        nc.vector.tensor_scalar(
                            acc, X[:, dj * S + PAD : dj * S + PAD + SEQ], w0, None,
                            mybir.AluOpType.mult,
                        )
                        for k in range(1, KERNEL):
                            wk = wst[:, hl * KERNEL + k : hl * KERNEL + k + 1]
                            nc.vector.scalar_tensor_tensor(
                                acc, X[:, dj * S + PAD - k : dj * S + PAD - k + SEQ], wk, acc,
                                mybir.AluOpType.mult, mybir.AluOpType.add,
                            )
                        nc.scalar.dma_start(out=out[:, dj, :], in_=acc)
    nc.compile()
    return nc


_module = None


def _get_module():
    global _module
    if _module is None:
        _module = _build_module()
    return _module


def _shard_inputs(x: np.ndarray, weight: np.ndarray):
    x16 = x.astype(np.float16)
    w16 = weight.astype(np.float16)
    in_maps = []
    for c in range(N_CORES):
        h0 = c * H_PER_CORE
        xs = x16[:, :, h0 : h0 + H_PER_CORE, :]              # (B, T, HL, D)
        xt = np.ascontiguousarray(xs.transpose(3, 2, 0, 1))  # (D, HL, B, T)
        xin = np.zeros((D_HEAD, N_STREAMS * S + SLACK), dtype=np.float16)
        view = xin[:, : N_STREAMS * S].reshape(D_HEAD, N_STREAMS, S)
        view[:, :, PAD:] = xt.reshape(D_HEAD, N_STREAMS, SEQ)

        ws_ = weight[:, h0 : h0 + H_PER_CORE, :]             # (K, HL, D) f32
        warr = np.ascontiguousarray(ws_.transpose(2, 1, 0)).reshape(
            D_HEAD, H_PER_CORE * KERNEL
        ).astype(np.float32)

        wdiag = np.zeros((D_HEAD, H_PER_CORE * KERNEL * D_HEAD), dtype=np.float16)
        for hl in range(H_PER_CORE):
            for k in range(KERNEL):
                blk = wdiag[:, (hl * KERNEL + k) * D_HEAD : (hl * KERNEL + k + 1) * D_HEAD]
                np.fill_diagonal(blk, w16[k, h0 + hl, :])
        in_maps.append({"x": xin, "wd": wdiag, "ws": warr})
    return in_maps


def _unshard(results) -> np.ndarray:
    out = np.empty((BATCH, SEQ, N_HEADS, D_HEAD), dtype=np.float32)
    for c in range(N_CORES):
        h0 = c * H_PER_CORE
        o = results[c]["out"].astype(np.float32).reshape(D_HEAD, H_PER_CORE, BATCH, SEQ)
        out[:, :, h0 : h0 + H_PER_CORE, :] = o.transpose(2, 3, 1, 0)
    return out


def kernel(x: np.ndarray, weight: np.ndarray) -> np.ndarray:
    global last_exec_time_ns, last_results
    x = np.asarray(x, dtype=np.float32)
    weight = np.asarray(weight, dtype=np.float32)
    nc = _get_module()
    in_maps = _shard_inputs(x, weight)
    # The shared terminal occasionally wedges (NRT_EXEC_UNIT_UNRECOVERABLE)
    # and recovers after a pause; retry rather than fail the whole call.
    last_err = None
    for attempt in range(3):
        try:
            res = run_bass_kernel_spmd(
                nc, in_maps, list(range(N_CORES)), trace=PROFILE, **TRACE_KWARGS
            )
            break
        except Exception as e:  # noqa: BLE001 - device-transient errors
            last_err = e
            time.sleep(25 * (attempt + 1))
    else:
        raise last_err
    last_exec_time_ns = res.exec_time_ns
    last_results = res
    return _unshard(res.results)


# revision 4
# speedup vs baseline: 1.4364x; 1.1159x over previous
"""Causal depthwise conv (kernel_size=4) on 8 TRN2 NeuronCores — fp16 hybrid.

Problem: x (4, 4096, 16, 128) f32, weight (4, 16, 128) f32,
out[b,t,h,d] = sum_k weight[k,h,d] * x[b,t-k,h,d]   (zero-pad t<0).

Sharding: tensor-parallel over heads — core c owns heads [2c, 2c+2), giving
8 streams per core (stream j = hl*BATCH + b), each a [d=128, t=4096] lane.

The kernel is HBM-DMA-bound, so all device I/O is fp16 (host casts both
ways; rel-err ~1e-3 vs the 2e-2 gate): 16.8 MB per core, ~39 us at the
~435 GB/s SBUF-fabric rate. All DMAs ride one HWDGE ring (SP) so reads and
writes form direction-coherent bursts — splitting them across the SP+ACT
rings measured ~278 GB/s from HBM R/W turnaround thrash.

Compute is split so every engine stays at or below the DMA wall:
  PE streams (5): each tap k is a matmul with a 128x128 *diagonal*
    stationary diag(w[k,h,:]) against the shifted moving slice x[d, t-k],
    accumulating all 4 taps into a PSUM bank per 512 cols; ScalarE evicts
    2048-col spans PSUM->SBUF as fp16. PE ~38 us, ACT ~29 us.
  DVE streams (3): acc = x*w0 (tensor_scalar), then 3 scalar_tensor_tensor
    taps; even-offset taps pack 2x, odd offsets run 1x. ~40 us.

Each stream ships with 4 leading zero columns (causal pad + even
alignment), so taps never read across stream boundaries and every DMA row
is a contiguous 8.2 KB run.
"""

import time

import numpy as np

import concourse.mybir as mybir
from concourse import bacc, tile
from concourse.bass_utils import run_bass_kernel_spmd

BATCH, SEQ, N_HEADS, D_HEAD = 4, 4096, 16, 128
KERNEL = 4
PAD = 4                                   # causal pad (>=KERNEL-1), even
N_CORES = 8
H_PER_CORE = N_HEADS // N_CORES           # 2
N_STREAMS = H_PER_CORE * BATCH            # 8 per core; stream j = hl*BATCH + b
S = SEQ + PAD                             # per-stream length (causal pad incl.)

DVE_STREAMS = (1, 3, 5)                   # interleaved with PE streams by DMA arrival
BANK = 512                                # one PSUM bank in f32 cols
EVICT = 2048                              # ACT eviction span (4 banks)

F32 = mybir.dt.float32
F16 = mybir.dt.float16

PROFILE = False          # set by test.py; adds a profiled run
TRACE_KWARGS = {}
last_exec_time_ns = None
last_results = None


def _build_module(chain: bool = False, repeats: int = 1):
    """repeats>1 runs the whole kernel body that many times inside one NEFF
    (timing only). `chain` is accepted for test.py compat (unused)."""
    nc = bacc.Bacc(
        "TRN2",
        target_bir_lowering=False,
        debug=False,
        num_devices=N_CORES,
        enable_asserts=False,
    )
    x = nc.dram_tensor("x", [D_HEAD, N_STREAMS, S], F16, kind="ExternalInput").ap()
    wd = nc.dram_tensor(
        "wd", [D_HEAD, H_PER_CORE * KERNEL * D_HEAD], F16, kind="ExternalInput"
    ).ap()
    ws = nc.dram_tensor("ws", [D_HEAD, H_PER_CORE * KERNEL], F32, kind="ExternalInput").ap()
    out = nc.dram_tensor("out", [D_HEAD, N_STREAMS, SEQ], F16, kind="ExternalOutput").ap()

    with tile.TileContext(nc) as tc:
        with (
            tc.tile_pool(name="wp", bufs=1) as wp,
            tc.tile_pool(name="xp", bufs=10) as xp,
            tc.tile_pool(name="accp", bufs=3) as accp,
            tc.tile_pool(name="outp", bufs=4) as outp,
            tc.psum_pool(name="pp", bufs=2) as pp,
        ):
            wdt = wp.tile([D_HEAD, H_PER_CORE * KERNEL * D_HEAD], F16)
            wst = wp.tile([D_HEAD, H_PER_CORE * KERNEL], F32)
            nc.sync.dma_start(out=wdt, in_=wd)
            nc.sync.dma_start(out=wst, in_=ws)

            for _r in range(repeats):
                # all input DMAs up front, one per stream, on the SP ring
                xt = []
                for j in range(N_STREAMS):
                    t = xp.tile([D_HEAD, S], F16, tag="x")
                    nc.sync.dma_start(out=t, in_=x[:, j, :])
                    xt.append(t)

                outs = []       # (dma issue thunks) in compute-issue order
                dve_done = 0
                for j in (0, 2, 4, 6, 7):          # PE streams
                    hl = j // BATCH
                    X = xt[j]
                    for half in range(SEQ // EVICT):
                        ps = pp.tile([D_HEAD, EVICT], F32, tag="ps")
                        for k in range(KERNEL):
                            wk = wdt[:, (hl * KERNEL + k) * D_HEAD : (hl * KERNEL + k + 1) * D_HEAD]
                            for c in range(EVICT // BANK):
                                base = PAD - k + half * EVICT + c * BANK
                                nc.tensor.matmul(
                                    ps[:, c * BANK : (c + 1) * BANK],
                                    wk,
                                    X[:, base : base + BANK],
                                    start=(k == 0),
                                    stop=(k == KERNEL - 1),
                                )
                        osb = outp.tile([D_HEAD, EVICT], F16, tag="osb")
                        nc.scalar.activation(
                            osb, ps, mybir.ActivationFunctionType.Copy, scale=1.0
                        )
                        nc.sync.dma_start(
                            out=out[:, j, half * EVICT : (half + 1) * EVICT], in_=osb
                        )
                    # interleave one DVE stream after each PE stream
                    if dve_done < len(DVE_STREAMS):
                        dj = DVE_STREAMS[dve_done]
                        dve_done += 1
                        hl = dj // BATCH
                        Xd = xt[dj]
                        acc = accp.tile([D_HEAD, SEQ], F16, tag="acc")
                        w0 = wst[:, hl * KERNEL : hl * KERNEL + 1]
                        nc.vector.tensor_scalar(
                            acc, Xd[:, PAD : PAD + SEQ], w0, None,
                            mybir.AluOpType.mult,
                        )
                        for k in range(1, KERNEL):
                            wk = wst[:, hl * KERNEL + k : hl * KERNEL + k + 1]
                            nc.vector.scalar_tensor_tensor(
                                acc, Xd[:, PAD - k : PAD - k + SEQ], wk, acc,
                                mybir.AluOpType.mult, mybir.AluOpType.add,
                            )
                        nc.sync.dma_start(out=out[:, dj, :], in_=acc)
    nc.compile()
    return nc


_module = None


def _get_module():
    global _module
    if _module is None:
        _module = _build_module()
    return _module


def _shard_inputs(x: np.ndarray, weight: np.ndarray):
    x16 = x.astype(np.float16)
    w16 = weight.astype(np.float16)
    in_maps = []
    for c in range(N_CORES):
        h0 = c * H_PER_CORE
        xs = x16[:, :, h0 : h0 + H_PER_CORE, :]              # (B, T, HL, D)
        xt = np.ascontiguousarray(xs.transpose(3, 2, 0, 1))  # (D, HL, B, T)
        xin = np.zeros((D_HEAD, N_STREAMS, S), dtype=np.float16)
        xin[:, :, PAD:] = xt.reshape(D_HEAD, N_STREAMS, SEQ)

        ws_ = weight[:, h0 : h0 + H_PER_CORE, :]             # (K, HL, D) f32
        warr = np.ascontiguousarray(ws_.transpose(2, 1, 0)).reshape(
            D_HEAD, H_PER_CORE * KERNEL
        ).astype(np.float32)

        wdiag = np.zeros((D_HEAD, H_PER_CORE * KERNEL * D_HEAD), dtype=np.float16)
        for hl in range(H_PER_CORE):
            for k in range(KERNEL):
                blk = wdiag[:, (hl * KERNEL + k) * D_HEAD : (hl * KERNEL + k + 1) * D_HEAD]
                np.fill_diagonal(blk, w16[k, h0 + hl, :])
        in_maps.append({"x": xin, "wd": wdiag, "ws": warr})
    return in_maps


def _unshard(results) -> np.ndarray:
    out = np.empty((BATCH, SEQ, N_HEADS, D_HEAD), dtype=np.float32)
    for c in range(N_CORES):
        h0 = c * H_PER_CORE
        o = results[c]["out"].astype(np.float32).reshape(D_HEAD, H_PER_CORE, BATCH, SEQ)
        out[:, :, h0 : h0 + H_PER_CORE, :] = o.transpose(2, 3, 1, 0)
    return out


def kernel(x: np.ndarray, weight: np.ndarray) -> np.ndarray:
    global last_exec_time_ns, last_results
    x = np.asarray(x, dtype=np.float32)
    weight = np.asarray(weight, dtype=np.float32)
    nc = _get_module()
    in_maps = _shard_inputs(x, weight)
    # The shared terminal occasionally wedges (NRT_EXEC_UNIT_UNRECOVERABLE)
    # and recovers after a pause; retry rather than fail the whole call.
    last_err = None
    for attempt in range(3):
        try:
            res = run_bass_kernel_spmd(
                nc, in_maps, list(range(N_CORES)), trace=PROFILE, **TRACE_KWARGS
            )
            break
        except Exception as e:  # noqa: BLE001 - device-transient errors
            last_err = e
            time.sleep(25 * (attempt + 1))
    else:
        raise last_err
    last_exec_time_ns = res.exec_time_ns
    last_results = res
    return _unshard(res.results)
